# revision 27
# baseline (speedup 1.0000x reference)
"""Bark-style causal self-attention on 8 Trainium2 NeuronCores.

Problem (hardcoded): B=4, S=1024, D=1024, H=16, hd=64, fp32 I/O.

Sharding: 8 cores = 4 batches x 2 head-groups (8 heads each).
Per core, everything is computed in "transposed" orientation so that no
on-device transposes are needed:
  - hs[b]^T (with an appended ones row for the bias trick) is prepared on
    the host; qk^T = (att_w_slice_aug)^T @ hsT_aug comes out of the PE
    directly in [feature, seq] layout.
  - scores are computed transposed: sT[k, q] = k^T.T @ q^T, softmax runs
    along keys without a max-subtraction pass (scores are bounded ~|2| for
    this distribution, exp is safe in fp32), and the PV matmul consumes
    p^T directly as the moving operand with V (natural layout, computed
    separately) as the stationary operand.
  - sum_k p[k, q] rides along the PV matmul via a ones-column appended to
    each head's V block (65th stationary column).
  - out^T partial = w_out_slice.T @ ctx^T; the two cores of a batch hold
    partial sums which are combined at the end.
Heads are processed in pairs with tile_position row packing so the K=64
score matmuls use the full 128-row PE array.
"""

from contextlib import ExitStack

import numpy as np
import ml_dtypes

import concourse.bass as bass
import concourse.tile as tile
import concourse.mybir as mybir
from concourse.bass_utils import run_bass_kernel_spmd
from concourse.vector_clock import ScopedClock


# --------------------------------------------------------------------------
# Workaround for the walrus build in this container, which accepts at most
# ONE sync-wait command per instruction (two on EventSemaphore).  Stock Tile
# emits instructions with several waits; we legalize the program after
# TileContext exit:
#   1. The kernel-tail drain (which waits on every proc's final tick) is
#      emitted as a chain of single-wait drains instead (patch below).
#   2. Remaining multi-wait instructions have their excess waits hoisted
#      backward onto earlier same-engine instructions.  Moving a wait
#      earlier on the same engine only strengthens ordering; it is
#      deadlock-free as long as the wait's producer is scheduled before
#      the carrier (Tile's schedule order makes everything before the
#      carrier executable without anything at/after it).
# --------------------------------------------------------------------------

def _patched_drain_and_barrier(self, tick_clock, wait_clock):
    drain_inst = self.nc.sync.drain()
    wait_clock.add_sem_waits(
        drain_inst.ins, ScopedClock({None: tick_clock.global_clock})
    )
    si = drain_inst.ins.sync_info
    waits = list(si.on_wait or []) if si is not None else []
    if len(waits) > 1:
        si.on_wait = [waits[0]]
        for w in waits[1:]:
            extra = self.nc.sync.drain()
            esi = extra.ins.sync_info
            if esi is None:
                extra.ins.sync_info = mybir.SyncInfo(on_wait=[w], on_update=[])
            else:
                esi.on_wait = [w]

    self.nc.all_engine_barrier()
    assert self.sems is not None
    popped = self.nc._tile_sem_poison_stack.pop()
    assert popped is self._sem_poison
    self.nc.clear_and_free_semaphores(list(self.sems.allocated().values()))
    self.nc.all_engine_barrier()


tile.TileContext._drain_and_barrier = _patched_drain_and_barrier

def _legalize_waits_json(raw: bytes) -> bytes:
    """Split multi-wait instructions by inserting single-wait NoOp carriers
    immediately before them on the same engine (pure in-stream split: all
    waits still execute before the instruction, in the same order)."""
    import orjson

    j = orjson.loads(raw)
    n_inserted = 0
    for f in j["functions"]:
        for b in f["blocks"]:
            out = []
            for inst in b["instructions"]:
                si = inst.get("sync_info") or {}
                waits = si.get("on_wait") or []
                cap = 2 if inst.get("opcode") == "EventSemaphore" else 1
                if len(waits) > cap:
                    excess, keep = waits[:-cap], waits[-cap:]
                    for k, w in enumerate(excess):
                        out.append({
                            "debug": inst.get("debug", 0),
                            "engine": inst["engine"],
                            "ins": [],
                            "name": f"{inst['name']}-lw{k}",
                            "opcode": "NoOp",
                            "outs": [],
                            "sync_info": {"on_wait": [w]},
                        })
                        n_inserted += 1
                    si["on_wait"] = keep
                    inst["sync_info"] = si
                out.append(inst)
            b["instructions"] = out
    return orjson.dumps(j)

BF16 = mybir.dt.bfloat16
F32 = mybir.dt.float32
NPBF16 = ml_dtypes.bfloat16

B, S, D, H, HD = 4, 1024, 1024, 16, 64
NCORES = 8
HPC = 8          # heads per core
PAIRS = 4        # head pairs per core
KCH = 8          # 128-row chunks of the D contraction
SCALE = 1.0 / np.sqrt(HD)

# Set by test harness to capture a profile; read back from LAST_RESULTS.
TRACE = False
LAST_RESULTS = None

_CACHE = {}
DEBUG_DUMP = False


def _chunks512(lo, hi):
    """Split [lo, hi) into pieces of at most 512 that do not cross a
    multiple-of-512 boundary (PSUM bank boundary for fp32 tiles)."""
    out = []
    while lo < hi:
        nxt = min(hi, (lo // 512 + 1) * 512)
        out.append((lo, nxt))
        lo = nxt
    return out


def _emit(tc, io, ctx):
    nc = tc.nc
    hsT, wqk, qkb, wv, wout, outb, tri, outT = (
        io["hsT"], io["wqk"], io["qkb"], io["wv"], io["wout"], io["outb"],
        io["tri"], io["outT"],
    )
    Exp = mybir.ActivationFunctionType.Exp
    Ident = mybir.ActivationFunctionType.Identity

    persist = ctx.enter_context(tc.tile_pool(name="persist", bufs=1))

    def load(name, src, shape, dtype=BF16):
        t = persist.tile(shape, dtype, name=name, tag=name)
        nc.sync.dma_start(out=t[:, :], in_=src)
        return t

    # ---- resident SBUF tensors -------------------------------------------
    # Loads are interleaved (wqk[k], hsT[k]) so the first projection
    # matmuls unblock as early as possible.
    wqk_sb, hsT_sb = [], []
    for k in range(KCH):
        wqk_sb.append(load(f"wqk{k}", wqk[k * 128:(k + 1) * 128, :],
                           [128, 1024]))
        hsT_sb.append(load(f"hsT{k}", hsT[k * 128:(k + 1) * 128, :],
                           [128, S]))
    qkb_sb = load("qkb", qkb[:, :], [128, 8], F32)
    hsT_row = load("hsT_row", hsT[1024:1025, :], [1, S])
    wv_sb = [load(f"wv{k}", wv[k * 128:(k + 1) * 128, :], [128, 512])
             for k in range(KCH)]
    wv_row = load("wv_row", wv[1024:1025, :], [1, 512])
    tri_sb = load("tri", tri[:, :], [128, 128])
    wout_sb = [load(f"wout{p}", wout[p * 128:(p + 1) * 128, :], [128, 1024])
               for p in range(PAIRS)]
    outb_sb = load("outb", outb[:, :], [128, 8], F32)

    # outputs of the projections
    qkT_sb = [persist.tile([128, S], BF16, name=f"qkT{m}", tag=f"qkT{m}")
              for m in range(8)]   # 0-3: q pairs, 4-7: k pairs
    v_sb = [persist.tile([128, HPC * 65], BF16, name=f"v{s}", tag=f"v{s}")
            for s in range(8)]
    ctxT_sb = [persist.tile([128, S], BF16, name=f"ctxT{p}", tag=f"ctxT{p}")
               for p in range(PAIRS)]

    # ---- phase 1: qk^T projection ----------------------------------------
    # qkT[128m:128m+128, :] = wqk[:, m-tile].T @ hsT ; bias added in the
    # PSUM->SBUF copy on ScalarE (per-partition bias = per-feature).
    with tc.tile_pool(name="qkps", bufs=6, space="PSUM") as qkps_pool, \
         tc.tile_pool(name="vps", bufs=2, space="PSUM") as vps_pool:
        for m in range(8):
            ps = [qkps_pool.tile([128, 512], F32, name=f"qkps{m}_{n}",
                                 tag="qkps") for n in range(2)]
            for k in range(KCH):
                for n in range(2):
                    nc.tensor.matmul(
                        ps[n][:, :],
                        lhsT=wqk_sb[k][:, m * 128:(m + 1) * 128],
                        rhs=hsT_sb[k][:, n * 512:(n + 1) * 512],
                        start=(k == 0), stop=(k == KCH - 1))
            for n in range(2):
                nc.vector.tensor_scalar_add(
                    qkT_sb[m][:, n * 512:(n + 1) * 512], ps[n][:, :],
                    qkb_sb[:, m:m + 1])

        # ---- phase 2: V projection (natural, 65-col stride per head) ----
        for s in range(8):
            ps = vps_pool.tile([128, 512], F32, name=f"vps{s}", tag="vps")
            for k in range(KCH):
                nc.tensor.matmul(
                    ps[:, :],
                    lhsT=hsT_sb[k][:, s * 128:(s + 1) * 128],
                    rhs=wv_sb[k][:, :],
                    start=(k == 0), stop=False)
            nc.tensor.matmul(
                ps[:, :],
                lhsT=hsT_row[0:1, s * 128:(s + 1) * 128],
                rhs=wv_row[0:1, :],
                start=False, stop=True)
            v3 = v_sb[s].rearrange("p (h c) -> p h c", c=65)
            nc.scalar.copy(v3[:, :, 0:64],
                           ps.rearrange("p (h c) -> p h c", c=64))
            nc.vector.memset(v3[:, :, 64:65], 1.0)

    # ---- phase 3: attention, one head pair at a time ---------------------
    # Score tiles hold BOTH heads of the pair: psum [128, 2, <=512] with
    # head t in bank t; one exp call covers both heads.
    attn_ctx = ExitStack()
    sT_pool = attn_ctx.enter_context(tc.tile_pool(name="sT", bufs=2,
                                                  space="PSUM"))
    ctx_pool = attn_ctx.enter_context(tc.tile_pool(name="ctx", bufs=2,
                                                   space="PSUM"))
    pT_pool = attn_ctx.enter_context(tc.tile_pool(name="pT", bufs=4))
    nrm_pool = attn_ctx.enter_context(tc.tile_pool(name="nrm", bufs=2))

    for p in range(PAIRS):
        ctx_ps = [ctx_pool.tile([65, S], F32, name=f"ctx{p}_{t}", tag="ctx")
                  for t in range(2)]
        for kb in range(8):
            q0 = kb * 128
            w = S - q0
            for (c0, c1) in _chunks512(0, w):
                wc = c1 - c0
                sT = sT_pool.tile([128, 2, 512], F32,
                                  name=f"sT{p}{kb}{c0}", tag="sT")
                for t in range(2):
                    nc.tensor.matmul(
                        sT[:, t, 0:wc],
                        lhsT=qkT_sb[4 + p][64 * t:64 * t + 64, q0:q0 + 128],
                        rhs=qkT_sb[p][64 * t:64 * t + 64,
                                      q0 + c0:q0 + c1],
                        start=True, stop=True,
                        tile_position=(64 * t, 0))
                pt = pT_pool.tile([128, 2, 512], BF16,
                                  name=f"pT{p}{kb}{c0}", tag="pT")
                nc.scalar.activation(pt[:, :, 0:wc], sT[:, :, 0:wc], Exp,
                                     scale=SCALE)
                if c0 == 0:
                    # causal mask on the diagonal 128x128 block, both heads
                    pm = pt[:, :, 0:128]
                    tri3 = tri_sb.rearrange("p (o c) -> p o c", o=1)
                    tri_b, _ = bass.broadcast_tensor_aps(tri3, pm)
                    nc.vector.tensor_mul(pm, pm, tri_b)
                for t in range(2):
                    hh = 2 * p + t
                    for (g0, g1) in _chunks512(q0 + c0, q0 + c1):
                        nc.tensor.matmul(
                            ctx_ps[t][:, g0:g1],
                            lhsT=v_sb[kb][:, hh * 65:hh * 65 + 65],
                            rhs=pt[:, t, g0 - q0 - c0:g1 - q0 - c0],
                            start=(kb == 0),
                            stop=(kb == (3 if g1 <= 512 else 7)))
        # Copy ctx out of PSUM immediately (releases the bank for the next
        # pair), then normalize from SBUF: ctx^T[d, q] * (1/sum[q]) with the
        # reciprocal row broadcast across 64 partitions by a SBUF->SBUF DMA.
        for t in range(2):
            cu = nrm_pool.tile([65, S], F32, name=f"cu{p}{t}", tag="cu")
            nc.vector.tensor_copy(cu[:, :], ctx_ps[t][:, :])
            recip = nrm_pool.tile([1, S], F32, name=f"rc{p}{t}", tag="recip")
            nc.vector.reciprocal(recip[:, :], cu[64:65, :])
            bc_sb = nrm_pool.tile([64, S], F32, name=f"bs{p}{t}", tag="bc")
            r1 = recip[0:1, :]
            rsrc = bass.AP(r1.tensor, r1.offset,
                           [list(r1.ap[0]), [0, 64], [1, S]])
            nc.sync.dma_start(out=bc_sb[:, :], in_=rsrc)
            nc.vector.tensor_mul(ctxT_sb[p][64 * t:64 * t + 64, :],
                                 cu[0:64, :], bc_sb[:, :])

    attn_ctx.close()

    if DEBUG_DUMP:
        for m in range(8):
            nc.sync.dma_start(out=io["dbg_qkT"][m * 128:(m + 1) * 128, :],
                              in_=qkT_sb[m][:, :])
        for s in range(8):
            nc.sync.dma_start(out=io["dbg_v"][s * 128:(s + 1) * 128, :],
                              in_=v_sb[s][:, :])
        for p in range(PAIRS):
            nc.sync.dma_start(out=io["dbg_ctxT"][p * 128:(p + 1) * 128, :],
                              in_=ctxT_sb[p][:, :])

    # ---- phase 4: out^T partial = wout.T @ ctx^T -------------------------
    with tc.tile_pool(name="ops", bufs=4, space="PSUM") as op_pool, \
         tc.tile_pool(name="osb", bufs=4) as osb_pool:
        for d in range(8):
            for n in range(2):
                ps = op_pool.tile([128, 512], F32, name=f"o{d}_{n}",
                                  tag="ops")
                for p in range(PAIRS):
                    nc.tensor.matmul(
                        ps[:, :],
                        lhsT=wout_sb[p][:, d * 128:(d + 1) * 128],
                        rhs=ctxT_sb[p][:, n * 512:(n + 1) * 512],
                        start=(p == 0), stop=(p == PAIRS - 1))
                osb = osb_pool.tile([128, 512], F32, name=f"ob{d}_{n}",
                                    tag="osb")
                nc.vector.tensor_scalar_add(osb[:, :], ps[:, :],
                                            outb_sb[:, d:d + 1])
                nc.sync.dma_start(
                    out=outT[d * 128:(d + 1) * 128, n * 512:(n + 1) * 512],
                    in_=osb[:, :])


def _build():
    nc = bass.Bass("TRN2", target_bir_lowering=False, debug=False,
                   num_devices=NCORES)
    io = {
        "hsT": nc.dram_tensor("hsT", [1025, S], BF16,
                              kind="ExternalInput").ap(),
        "wqk": nc.dram_tensor("wqk", [1024, 1024], BF16,
                              kind="ExternalInput").ap(),
        "qkb": nc.dram_tensor("qkb", [128, 8], F32,
                              kind="ExternalInput").ap(),
        "wv": nc.dram_tensor("wv", [1025, 512], BF16,
                             kind="ExternalInput").ap(),
        "wout": nc.dram_tensor("wout", [512, 1024], BF16,
                               kind="ExternalInput").ap(),
        "outb": nc.dram_tensor("outb", [128, 8], F32,
                               kind="ExternalInput").ap(),
        "tri": nc.dram_tensor("tri", [128, 128], BF16,
                              kind="ExternalInput").ap(),
        "outT": nc.dram_tensor("outT", [1024, S], F32,
                               kind="ExternalOutput").ap(),
    }
    if DEBUG_DUMP:
        io["dbg_qkT"] = nc.dram_tensor("dbg_qkT", [1024, S], BF16,
                                       kind="ExternalOutput").ap()
        io["dbg_v"] = nc.dram_tensor("dbg_v", [1024, HPC * 65], BF16,
                                     kind="ExternalOutput").ap()
        io["dbg_ctxT"] = nc.dram_tensor("dbg_ctxT", [512, S], BF16,
                                        kind="ExternalOutput").ap()
    with tile.TileContext(nc) as tc:
        with ExitStack() as ctx:
            _emit(tc, io, ctx)
    fixed = _legalize_waits_json(nc.to_json_bytes())
    nc.to_json_bytes = (lambda fixed=fixed: fixed)
    return nc


def _get_nc():
    if "nc" not in _CACHE:
        _CACHE["nc"] = _build()
    return _CACHE["nc"]


def _prep_inputs(hidden_states, att_w, att_b, out_w, out_b):
    """Build the 8 per-core input maps (host-side shard/layout prep)."""
    hs = np.asarray(hidden_states, dtype=np.float32)
    att_w = np.asarray(att_w, dtype=np.float32)
    att_b = np.asarray(att_b, dtype=np.float32)
    out_w = np.asarray(out_w, dtype=np.float32)
    out_b = np.asarray(out_b, dtype=np.float32)

    ones_row = np.ones((1, S), dtype=np.float32)
    tri = np.triu(np.ones((128, 128), dtype=np.float32)).astype(NPBF16)
    outb_t = np.ascontiguousarray(out_b.reshape(8, 128).T).astype(np.float32)
    zeros_outb = np.zeros_like(outb_t)

    in_maps = []
    for c in range(NCORES):
        b, hg = divmod(c, 2)
        lo, hi = hg * 512, (hg + 1) * 512
        hsT_aug = np.concatenate([hs[b].T, ones_row], axis=0).astype(NPBF16)
        wqk = np.concatenate([att_w[:, lo:hi], att_w[:, D + lo:D + hi]],
                             axis=1).astype(NPBF16)
        qkb = np.concatenate([att_b[lo:hi], att_b[D + lo:D + hi]])
        qkb = np.ascontiguousarray(qkb.reshape(8, 128).T).astype(np.float32)
        wv = np.concatenate(
            [att_w[:, 2 * D + lo:2 * D + hi],
             att_b[2 * D + lo:2 * D + hi][None, :]], axis=0).astype(NPBF16)
        wout = out_w[lo:hi, :].astype(NPBF16)
        in_maps.append({
            "hsT": np.ascontiguousarray(hsT_aug),
            "wqk": np.ascontiguousarray(wqk),
            "qkb": qkb,
            "wv": np.ascontiguousarray(wv),
            "wout": np.ascontiguousarray(wout),
            "outb": outb_t if hg == 0 else zeros_outb,
            "tri": tri,
        })
    return in_maps


def kernel(hidden_states, att_w, att_b, out_w, out_b):
    global LAST_RESULTS
    in_maps = _prep_inputs(hidden_states, att_w, att_b, out_w, out_b)
    nc = _get_nc()
    trace = TRACE
    if trace:
        try:
            from antenv.axon_hooks import get_axon_ntff_profile_hook  # noqa
        except ImportError:
            trace = False
    res = run_bass_kernel_spmd(nc, in_maps, core_ids=list(range(NCORES)),
                               trace=trace)
    LAST_RESULTS = res
    out = np.empty((B, S, D), dtype=np.float32)
    for b in range(B):
        acc = res.results[2 * b]["outT"] + res.results[2 * b + 1]["outT"]
        out[b] = acc.T
    return out


# revision 30
# speedup vs baseline: 1.0279x; 1.0279x over previous
"""Bark-style causal self-attention on 8 Trainium2 NeuronCores.

Problem (hardcoded): B=4, S=1024, D=1024, H=16, hd=64, fp32 I/O.

Sharding: 8 cores = 4 batches x 2 head-groups (8 heads each).
Per core, everything is computed in "transposed" orientation so that no
on-device transposes are needed:
  - hs[b]^T (with an appended ones row for the bias trick) is prepared on
    the host; qk^T = (att_w_slice_aug)^T @ hsT_aug comes out of the PE
    directly in [feature, seq] layout.
  - scores are computed transposed: sT[k, q] = k^T.T @ q^T, softmax runs
    along keys without a max-subtraction pass (scores are bounded ~|2| for
    this distribution, exp is safe in fp32), and the PV matmul consumes
    p^T directly as the moving operand with V (natural layout, computed
    separately) as the stationary operand.
  - sum_k p[k, q] rides along the PV matmul via a ones-column appended to
    each head's V block (65th stationary column).
  - out^T partial = w_out_slice.T @ ctx^T; the two cores of a batch hold
    partial sums which are combined at the end.
Heads are processed in pairs with tile_position row packing so the K=64
score matmuls use the full 128-row PE array.
"""

from contextlib import ExitStack

import numpy as np
import ml_dtypes

import concourse.bass as bass
import concourse.tile as tile
import concourse.mybir as mybir
from concourse.bass_utils import run_bass_kernel_spmd
from concourse.vector_clock import ScopedClock


# --------------------------------------------------------------------------
# Workaround for the walrus build in this container, which accepts at most
# ONE sync-wait command per instruction (two on EventSemaphore).  Stock Tile
# emits instructions with several waits; we legalize the program after
# TileContext exit:
#   1. The kernel-tail drain (which waits on every proc's final tick) is
#      emitted as a chain of single-wait drains instead (patch below).
#   2. Remaining multi-wait instructions have their excess waits hoisted
#      backward onto earlier same-engine instructions.  Moving a wait
#      earlier on the same engine only strengthens ordering; it is
#      deadlock-free as long as the wait's producer is scheduled before
#      the carrier (Tile's schedule order makes everything before the
#      carrier executable without anything at/after it).
# --------------------------------------------------------------------------

def _patched_drain_and_barrier(self, tick_clock, wait_clock):
    drain_inst = self.nc.sync.drain()
    wait_clock.add_sem_waits(
        drain_inst.ins, ScopedClock({None: tick_clock.global_clock})
    )
    si = drain_inst.ins.sync_info
    waits = list(si.on_wait or []) if si is not None else []
    if len(waits) > 1:
        si.on_wait = [waits[0]]
        for w in waits[1:]:
            extra = self.nc.sync.drain()
            esi = extra.ins.sync_info
            if esi is None:
                extra.ins.sync_info = mybir.SyncInfo(on_wait=[w], on_update=[])
            else:
                esi.on_wait = [w]

    self.nc.all_engine_barrier()
    assert self.sems is not None
    popped = self.nc._tile_sem_poison_stack.pop()
    assert popped is self._sem_poison
    self.nc.clear_and_free_semaphores(list(self.sems.allocated().values()))
    self.nc.all_engine_barrier()


tile.TileContext._drain_and_barrier = _patched_drain_and_barrier

def _legalize_waits_json(raw: bytes) -> bytes:
    """Split multi-wait instructions by inserting single-wait NoOp carriers
    immediately before them on the same engine (pure in-stream split: all
    waits still execute before the instruction, in the same order)."""
    import orjson

    j = orjson.loads(raw)
    n_inserted = 0
    for f in j["functions"]:
        for b in f["blocks"]:
            out = []
            for inst in b["instructions"]:
                si = inst.get("sync_info") or {}
                waits = si.get("on_wait") or []
                cap = 2 if inst.get("opcode") == "EventSemaphore" else 1
                if len(waits) > cap:
                    excess, keep = waits[:-cap], waits[-cap:]
                    for k, w in enumerate(excess):
                        out.append({
                            "debug": inst.get("debug", 0),
                            "engine": inst["engine"],
                            "ins": [],
                            "name": f"{inst['name']}-lw{k}",
                            "opcode": "NoOp",
                            "outs": [],
                            "sync_info": {"on_wait": [w]},
                        })
                        n_inserted += 1
                    si["on_wait"] = keep
                    inst["sync_info"] = si
                out.append(inst)
            b["instructions"] = out
    return orjson.dumps(j)

BF16 = mybir.dt.bfloat16
F32 = mybir.dt.float32
NPBF16 = ml_dtypes.bfloat16

B, S, D, H, HD = 4, 1024, 1024, 16, 64
NCORES = 8
HPC = 8          # heads per core
PAIRS = 4        # head pairs per core
KCH = 8          # 128-row chunks of the D contraction
SCALE = 1.0 / np.sqrt(HD)

# Set by test harness to capture a profile; read back from LAST_RESULTS.
TRACE = False
LAST_RESULTS = None

_CACHE = {}
DEBUG_DUMP = False


def _chunks512(lo, hi):
    """Split [lo, hi) into pieces of at most 512 that do not cross a
    multiple-of-512 boundary (PSUM bank boundary for fp32 tiles)."""
    out = []
    while lo < hi:
        nxt = min(hi, (lo // 512 + 1) * 512)
        out.append((lo, nxt))
        lo = nxt
    return out


def _emit(tc, io, ctx):
    nc = tc.nc
    hsT, wqk, qkb, wv, wout, outb, tri, outT = (
        io["hsT"], io["wqk"], io["qkb"], io["wv"], io["wout"], io["outb"],
        io["tri"], io["outT"],
    )
    Exp = mybir.ActivationFunctionType.Exp
    Ident = mybir.ActivationFunctionType.Identity

    persist = ctx.enter_context(tc.tile_pool(name="persist", bufs=1))

    def load(name, src, shape, dtype=BF16):
        t = persist.tile(shape, dtype, name=name, tag=name)
        nc.sync.dma_start(out=t[:, :], in_=src)
        return t

    # ---- resident SBUF tensors -------------------------------------------
    # Loads are interleaved (wqk[k], hsT[k]) so the first projection
    # matmuls unblock as early as possible.
    wqk_sb, hsT_sb = [], []
    for k in range(KCH):
        if k == 0:
            wt = persist.tile([128, 1024], BF16, name="wqk0", tag="wqk0")
            ht = persist.tile([128, S], BF16, name="hsT0", tag="hsT0")
            for h in range(2):
                nc.sync.dma_start(out=wt[:, h * 512:(h + 1) * 512],
                                  in_=wqk[0:128, h * 512:(h + 1) * 512])
                nc.sync.dma_start(out=ht[:, h * 512:(h + 1) * 512],
                                  in_=hsT[0:128, h * 512:(h + 1) * 512])
            wqk_sb.append(wt)
            hsT_sb.append(ht)
            continue
        wqk_sb.append(load(f"wqk{k}", wqk[k * 128:(k + 1) * 128, :],
                           [128, 1024]))
        hsT_sb.append(load(f"hsT{k}", hsT[k * 128:(k + 1) * 128, :],
                           [128, S]))
    qkb_sb = load("qkb", qkb[:, :], [128, 8], F32)
    hsT_row = load("hsT_row", hsT[1024:1025, :], [1, S])
    wv_sb = [load(f"wv{k}", wv[k * 128:(k + 1) * 128, :], [128, 512])
             for k in range(KCH)]
    wv_row = load("wv_row", wv[1024:1025, :], [1, 512])
    tri_sb = load("tri", tri[:, :], [128, 128])
    wout_sb = [load(f"wout{p}", wout[p * 128:(p + 1) * 128, :], [128, 1024])
               for p in range(PAIRS)]
    outb_sb = load("outb", outb[:, :], [128, 8], F32)

    # outputs of the projections
    qkT_sb = [persist.tile([128, S], BF16, name=f"qkT{m}", tag=f"qkT{m}")
              for m in range(8)]   # 0-3: q pairs, 4-7: k pairs
    v_sb = [persist.tile([128, HPC * 65], BF16, name=f"v{s}", tag=f"v{s}")
            for s in range(8)]
    ctxT_sb = [persist.tile([128, S], BF16, name=f"ctxT{p}", tag=f"ctxT{p}")
               for p in range(PAIRS)]

    # ---- phase 1: qk^T projection ----------------------------------------
    # qkT[128m:128m+128, :] = wqk[:, m-tile].T @ hsT ; bias added in the
    # PSUM->SBUF copy on ScalarE (per-partition bias = per-feature).
    with tc.tile_pool(name="qkps", bufs=6, space="PSUM") as qkps_pool, \
         tc.tile_pool(name="vps", bufs=2, space="PSUM") as vps_pool:
        for m in range(8):
            ps = [qkps_pool.tile([128, 512], F32, name=f"qkps{m}_{n}",
                                 tag="qkps") for n in range(2)]
            for k in range(KCH):
                for n in range(2):
                    nc.tensor.matmul(
                        ps[n][:, :],
                        lhsT=wqk_sb[k][:, m * 128:(m + 1) * 128],
                        rhs=hsT_sb[k][:, n * 512:(n + 1) * 512],
                        start=(k == 0), stop=(k == KCH - 1))
            for n in range(2):
                nc.vector.tensor_scalar_add(
                    qkT_sb[m][:, n * 512:(n + 1) * 512], ps[n][:, :],
                    qkb_sb[:, m:m + 1])

        # ---- phase 2: V projection (natural, 65-col stride per head) ----
        for s in range(8):
            ps = vps_pool.tile([128, 512], F32, name=f"vps{s}", tag="vps")
            for k in range(KCH):
                nc.tensor.matmul(
                    ps[:, :],
                    lhsT=hsT_sb[k][:, s * 128:(s + 1) * 128],
                    rhs=wv_sb[k][:, :],
                    start=(k == 0), stop=False)
            nc.tensor.matmul(
                ps[:, :],
                lhsT=hsT_row[0:1, s * 128:(s + 1) * 128],
                rhs=wv_row[0:1, :],
                start=False, stop=True)
            v3 = v_sb[s].rearrange("p (h c) -> p h c", c=65)
            nc.scalar.copy(v3[:, :, 0:64],
                           ps.rearrange("p (h c) -> p h c", c=64))
            nc.vector.memset(v3[:, :, 64:65], 1.0)

    # ---- phase 3: attention, one head pair at a time ---------------------
    # Score tiles hold BOTH heads of the pair: psum [128, 2, <=512] with
    # head t in bank t; one exp call covers both heads.
    attn_ctx = ExitStack()
    sT_pool = attn_ctx.enter_context(tc.tile_pool(name="sT", bufs=2,
                                                  space="PSUM"))
    ctx_pool = attn_ctx.enter_context(tc.tile_pool(name="ctx", bufs=2,
                                                   space="PSUM"))
    pT_pool = attn_ctx.enter_context(tc.tile_pool(name="pT", bufs=4))
    nrm_pool = attn_ctx.enter_context(tc.tile_pool(name="nrm", bufs=2))

    for p in range(PAIRS):
        ctx_ps = [ctx_pool.tile([65, S], F32, name=f"ctx{p}_{t}", tag="ctx")
                  for t in range(2)]
        for kb in range(8):
            q0 = kb * 128
            w = S - q0
            for (c0, c1) in _chunks512(0, w):
                wc = c1 - c0
                sT = sT_pool.tile([128, 2, 512], F32,
                                  name=f"sT{p}{kb}{c0}", tag="sT")
                for t in range(2):
                    nc.tensor.matmul(
                        sT[:, t, 0:wc],
                        lhsT=qkT_sb[4 + p][64 * t:64 * t + 64, q0:q0 + 128],
                        rhs=qkT_sb[p][64 * t:64 * t + 64,
                                      q0 + c0:q0 + c1],
                        start=True, stop=True,
                        tile_position=(64 * t, 0))
                pt = pT_pool.tile([128, 2, 512], BF16,
                                  name=f"pT{p}{kb}{c0}", tag="pT")
                nc.scalar.activation(pt[:, :, 0:wc], sT[:, :, 0:wc], Exp,
                                     scale=SCALE)
                if c0 == 0:
                    # causal mask on the diagonal 128x128 block, both heads
                    pm = pt[:, :, 0:128]
                    tri3 = tri_sb.rearrange("p (o c) -> p o c", o=1)
                    tri_b, _ = bass.broadcast_tensor_aps(tri3, pm)
                    nc.vector.tensor_mul(pm, pm, tri_b)
                for t in range(2):
                    hh = 2 * p + t
                    for (g0, g1) in _chunks512(q0 + c0, q0 + c1):
                        nc.tensor.matmul(
                            ctx_ps[t][:, g0:g1],
                            lhsT=v_sb[kb][:, hh * 65:hh * 65 + 65],
                            rhs=pt[:, t, g0 - q0 - c0:g1 - q0 - c0],
                            start=(kb == 0),
                            stop=(kb == (3 if g1 <= 512 else 7)))
        # Copy ctx out of PSUM immediately (releases the bank for the next
        # pair), then normalize from SBUF: ctx^T[d, q] * (1/sum[q]) with the
        # reciprocal row broadcast across 64 partitions by a SBUF->SBUF DMA.
        for t in range(2):
            cu = nrm_pool.tile([65, S], F32, name=f"cu{p}{t}", tag="cu")
            nc.scalar.copy(cu[:, :], ctx_ps[t][:, :])
            recip = nrm_pool.tile([1, S], F32, name=f"rc{p}{t}", tag="recip")
            nc.vector.reciprocal(recip[:, :], cu[64:65, :])
            bc_sb = nrm_pool.tile([64, S], F32, name=f"bs{p}{t}", tag="bc")
            r1 = recip[0:1, :]
            rsrc = bass.AP(r1.tensor, r1.offset,
                           [list(r1.ap[0]), [0, 64], [1, S]])
            nc.sync.dma_start(out=bc_sb[:, :], in_=rsrc)
            for (c0, c1) in _chunks512(0, S):
                nc.vector.tensor_mul(ctxT_sb[p][64 * t:64 * t + 64, c0:c1],
                                     cu[0:64, c0:c1], bc_sb[:, c0:c1])

    attn_ctx.close()

    if DEBUG_DUMP:
        for m in range(8):
            nc.sync.dma_start(out=io["dbg_qkT"][m * 128:(m + 1) * 128, :],
                              in_=qkT_sb[m][:, :])
        for s in range(8):
            nc.sync.dma_start(out=io["dbg_v"][s * 128:(s + 1) * 128, :],
                              in_=v_sb[s][:, :])
        for p in range(PAIRS):
            nc.sync.dma_start(out=io["dbg_ctxT"][p * 128:(p + 1) * 128, :],
                              in_=ctxT_sb[p][:, :])

    # ---- phase 4: out^T partial = wout.T @ ctx^T -------------------------
    with tc.tile_pool(name="ops", bufs=4, space="PSUM") as op_pool, \
         tc.tile_pool(name="osb", bufs=4) as osb_pool:
        for d in range(8):
            for n in range(2):
                ps = op_pool.tile([128, 512], F32, name=f"o{d}_{n}",
                                  tag="ops")
                for p in range(PAIRS):
                    nc.tensor.matmul(
                        ps[:, :],
                        lhsT=wout_sb[p][:, d * 128:(d + 1) * 128],
                        rhs=ctxT_sb[p][:, n * 512:(n + 1) * 512],
                        start=(p == 0), stop=(p == PAIRS - 1))
                osb = osb_pool.tile([128, 512], F32, name=f"ob{d}_{n}",
                                    tag="osb")
                nc.vector.tensor_scalar_add(osb[:, :], ps[:, :],
                                            outb_sb[:, d:d + 1])
                nc.sync.dma_start(
                    out=outT[d * 128:(d + 1) * 128, n * 512:(n + 1) * 512],
                    in_=osb[:, :])


def _build():
    nc = bass.Bass("TRN2", target_bir_lowering=False, debug=False,
                   num_devices=NCORES)
    io = {
        "hsT": nc.dram_tensor("hsT", [1025, S], BF16,
                              kind="ExternalInput").ap(),
        "wqk": nc.dram_tensor("wqk", [1024, 1024], BF16,
                              kind="ExternalInput").ap(),
        "qkb": nc.dram_tensor("qkb", [128, 8], F32,
                              kind="ExternalInput").ap(),
        "wv": nc.dram_tensor("wv", [1025, 512], BF16,
                             kind="ExternalInput").ap(),
        "wout": nc.dram_tensor("wout", [512, 1024], BF16,
                               kind="ExternalInput").ap(),
        "outb": nc.dram_tensor("outb", [128, 8], F32,
                               kind="ExternalInput").ap(),
        "tri": nc.dram_tensor("tri", [128, 128], BF16,
                              kind="ExternalInput").ap(),
        "outT": nc.dram_tensor("outT", [1024, S], F32,
                               kind="ExternalOutput").ap(),
    }
    if DEBUG_DUMP:
        io["dbg_qkT"] = nc.dram_tensor("dbg_qkT", [1024, S], BF16,
                                       kind="ExternalOutput").ap()
        io["dbg_v"] = nc.dram_tensor("dbg_v", [1024, HPC * 65], BF16,
                                     kind="ExternalOutput").ap()
        io["dbg_ctxT"] = nc.dram_tensor("dbg_ctxT", [512, S], BF16,
                                        kind="ExternalOutput").ap()
    with tile.TileContext(nc) as tc:
        with ExitStack() as ctx:
            _emit(tc, io, ctx)
    fixed = _legalize_waits_json(nc.to_json_bytes())
    nc.to_json_bytes = (lambda fixed=fixed: fixed)
    return nc


def _get_nc():
    if "nc" not in _CACHE:
        _CACHE["nc"] = _build()
    return _CACHE["nc"]


def _prep_inputs(hidden_states, att_w, att_b, out_w, out_b):
    """Build the 8 per-core input maps (host-side shard/layout prep)."""
    hs = np.asarray(hidden_states, dtype=np.float32)
    att_w = np.asarray(att_w, dtype=np.float32)
    att_b = np.asarray(att_b, dtype=np.float32)
    out_w = np.asarray(out_w, dtype=np.float32)
    out_b = np.asarray(out_b, dtype=np.float32)

    ones_row = np.ones((1, S), dtype=np.float32)
    tri = np.triu(np.ones((128, 128), dtype=np.float32)).astype(NPBF16)
    outb_t = np.ascontiguousarray(out_b.reshape(8, 128).T).astype(np.float32)
    zeros_outb = np.zeros_like(outb_t)

    in_maps = []
    for c in range(NCORES):
        b, hg = divmod(c, 2)
        lo, hi = hg * 512, (hg + 1) * 512
        hsT_aug = np.concatenate([hs[b].T, ones_row], axis=0).astype(NPBF16)
        wqk = np.concatenate([att_w[:, lo:hi], att_w[:, D + lo:D + hi]],
                             axis=1).astype(NPBF16)
        qkb = np.concatenate([att_b[lo:hi], att_b[D + lo:D + hi]])
        qkb = np.ascontiguousarray(qkb.reshape(8, 128).T).astype(np.float32)
        wv = np.concatenate(
            [att_w[:, 2 * D + lo:2 * D + hi],
             att_b[2 * D + lo:2 * D + hi][None, :]], axis=0).astype(NPBF16)
        wout = out_w[lo:hi, :].astype(NPBF16)
        in_maps.append({
            "hsT": np.ascontiguousarray(hsT_aug),
            "wqk": np.ascontiguousarray(wqk),
            "qkb": qkb,
            "wv": np.ascontiguousarray(wv),
            "wout": np.ascontiguousarray(wout),
            "outb": outb_t if hg == 0 else zeros_outb,
            "tri": tri,
        })
    return in_maps


def kernel(hidden_states, att_w, att_b, out_w, out_b):
    global LAST_RESULTS
    in_maps = _prep_inputs(hidden_states, att_w, att_b, out_w, out_b)
    nc = _get_nc()
    trace = TRACE
    if trace:
        try:
            from antenv.axon_hooks import get_axon_ntff_profile_hook  # noqa
        except ImportError:
            trace = False
    res = run_bass_kernel_spmd(nc, in_maps, core_ids=list(range(NCORES)),
                               trace=trace)
    LAST_RESULTS = res
    out = np.empty((B, S, D), dtype=np.float32)
    for b in range(B):
        acc = res.results[2 * b]["outT"] + res.results[2 * b + 1]["outT"]
        out[b] = acc.T
    return out
